# revision 1
# baseline (speedup 1.0000x reference)
"""Trainium2 Bass kernel for causal multi-head attention block.

Reference computation (B=4, S=2048, D=1024, H=16, HD=64, fp32):
    qkv = x @ Wqkv + bqkv; split q,k,v; per-head scaled scores;
    causal mask filled with -0.0001 (leaky, NOT -inf); softmax over all
    2048 keys; out = P @ V; out = out @ Wo + bo.

Sharding: 8 cores, core = (batch b = i//2, parity p = i%2). Each core
computes 1024 queries of its batch: query tiles t = 2j+p (j=0..3) of
256 queries. Causal block structure (512-key blocks per 256-query tile)
is then identical on every core: qtile j needs key blocks 0..j -> one
SPMD program, zero cross-core communication.

The leaky mask is handled exactly:
  - computed blocks: E' = exp(S)*M + (1-M)*w  with w = exp(-1e-4)
  - skipped key blocks (all masked): contribute w*SufV[j] to the
    numerator (suffix sums of V at 512-block granularity) and
    w*n_skip to the denominator Z.
Z is produced inside the PV matmul via a 65th all-ones V column.
Matmuls run as float32r (full-rate fp32 streaming); Q/K are stored
bf16 so the dh=64 score matmuls stream at full fetch rate.
"""

import math
from contextlib import ExitStack

import numpy as np

import concourse.bass as bass
import concourse.mybir as mybir
import concourse.tile as tile
from concourse import bacc

F32 = mybir.dt.float32
F32R = mybir.dt.float32r
BF16 = mybir.dt.bfloat16
AF = mybir.ActivationFunctionType
ALU = mybir.AluOpType
AX = mybir.AxisListType

B, S, D, H, HD = 4, 2048, 1024, 16, 64
QL, QT, KB, NJ = 1024, 256, 512, 4    # queries/core, qtile, key block, n qtiles
NCH = D // 128                         # contraction chunks
PAIRS = H // 2
W_MASK = math.exp(-1e-4)


def _r(ap):
    return ap


def build_program():
    nc = bacc.Bacc(
        "TRN2",
        target_bir_lowering=False,
        debug=False,
        num_devices=8,
    )
    xT = nc.declare_dram_parameter("xT", [D, S], F32R, isOutput=False)
    xqT = nc.declare_dram_parameter("xqT", [D, QL], F32R, isOutput=False)
    wqkv = nc.declare_dram_parameter("wqkv", [D, 3 * D], F32R, isOutput=False)
    wo = nc.declare_dram_parameter("wo", [D, D], F32R, isOutput=False)
    b2h = nc.declare_dram_parameter("b2h", [128, 24], F32, isOutput=False)
    brow = nc.declare_dram_parameter("brow", [1, 3 * D], F32R, isOutput=False)
    bv512 = nc.declare_dram_parameter("bv512", [128, 8], F32, isOutput=False)
    bocol = nc.declare_dram_parameter("bocol", [128, 8], F32, isOutput=False)
    mmul = nc.declare_dram_parameter("mmul", [128, 4 * QT], F32R, isOutput=False)
    madd = nc.declare_dram_parameter("madd", [128, 4 * QT], F32R, isOutput=False)
    onesd = nc.declare_dram_parameter("onesd", [128, 128], F32R, isOutput=False)
    outT = nc.declare_dram_parameter("outT", [D, QL], F32, isOutput=True)

    with tile.TileContext(nc) as tc, ExitStack() as ctx, \
         nc.allow_low_precision(reason="float32r matmul inputs are fp32 bits"):
        consts = ctx.enter_context(tc.tile_pool(name="consts", bufs=1))
        b2h_sb = consts.tile([128, 24], F32)
        nc.sync.dma_start(out=b2h_sb, in_=b2h[:])
        brow_sb = consts.tile([1, D], F32R)
        nc.sync.dma_start(out=brow_sb, in_=brow[0:1, 2 * D:3 * D])
        bv512_sb = consts.tile([128, 8], F32)
        nc.sync.dma_start(out=bv512_sb, in_=bv512[:])
        bocol_sb = consts.tile([128, 8], F32)
        nc.sync.dma_start(out=bocol_sb, in_=bocol[:])
        mmul_sb = consts.tile([128, 4 * QT], F32R)
        nc.sync.dma_start(out=mmul_sb, in_=mmul[:])
        madd_sb = consts.tile([128, 4 * QT], F32R)
        nc.sync.dma_start(out=madd_sb, in_=madd[:])
        ones_sb = consts.tile([1, 128], F32R)
        nc.sync.dma_start(out=ones_sb, in_=onesd[0:1, :])

        with ExitStack() as ctx2:
            xt_pool = ctx2.enter_context(tc.tile_pool(name="xt", bufs=1))
            xT_sb = xt_pool.tile([128, NCH, S], F32R)
            for c in range(NCH):
                nc.sync.dma_start(out=xT_sb[:, c, :], in_=xT[128 * c:128 * (c + 1), :])

            # per-512-block column sums of xT (for V block-sum corrections)
            xsum_sb = consts.tile([128, NCH, 4], F32R)
            for c in range(NCH):
                nc.vector.tensor_reduce(
                    out=xsum_sb[:, c, :],
                    in_=xT_sb[:, c, :].rearrange("p (b t) -> p b t", b=4),
                    axis=AX.X, op=ALU.add,
                )

            # ---------- Q projection, all head pairs up front ----------
            qt_pool = ctx2.enter_context(tc.tile_pool(name="qt", bufs=1))
            QT_all = qt_pool.tile([128, PAIRS, QL], BF16)
            with tc.tile_pool(name="xq", bufs=1) as xq_pool, \
                 tc.tile_pool(name="wq", bufs=2) as wq_pool, \
                 tc.tile_pool(name="qps", bufs=2, space="PSUM") as qps_pool:
                xqT_sb = xq_pool.tile([128, NCH, QL], F32R)
                for c in range(NCH):
                    nc.sync.dma_start(out=xqT_sb[:, c, :], in_=xqT[128 * c:128 * (c + 1), :])
                for pr in range(PAIRS):
                    wq_sb = wq_pool.tile([128, NCH, 128], F32R)
                    nc.sync.dma_start(
                        out=wq_sb,
                        in_=wqkv[:, 128 * pr:128 * (pr + 1)].rearrange("(c p) m -> p c m", p=128),
                    )
                    for g2 in range(2):
                        ps = qps_pool.tile([128, 512], F32)
                        for c in range(NCH):
                            nc.tensor.matmul(
                                out=ps, lhsT=_r(wq_sb[:, c, :]),
                                rhs=_r(xqT_sb[:, c, 512 * g2:512 * (g2 + 1)]),
                                start=(c == 0), stop=(c == NCH - 1),
                            )
                        # QT = (x@Wq)*0.125 + bq/8  (bias columns pre-divided on host)
                        nc.vector.tensor_scalar(
                            out=QT_all[:, pr, 512 * g2:512 * (g2 + 1)], in0=ps,
                            scalar1=0.125, scalar2=b2h_sb[:, pr:pr + 1],
                            op0=ALU.mult, op1=ALU.add,
                        )

            # ---------- main loop: 4 groups of 4 heads ----------
            vpool = ctx2.enter_context(tc.tile_pool(name="vsb", bufs=2))
            kt_pool = ctx2.enter_context(tc.tile_pool(name="kt", bufs=2))
            odram = ctx2.enter_context(tc.tile_pool(name="odram", bufs=1, space="DRAM"))
            O_dr = odram.tile([D, QL], F32R)       # [h*d, q] transposed head outputs

            with tc.tile_pool(name="wv", bufs=2) as wv_pool, \
                 tc.tile_pool(name="wk", bufs=2) as wk_pool, \
                 tc.tile_pool(name="pps", bufs=2, space="PSUM") as pps_pool, \
                 tc.tile_pool(name="sps", bufs=2, space="PSUM") as sps_pool, \
                 tc.tile_pool(name="ops", bufs=2, space="PSUM") as ops_pool, \
                 tc.tile_pool(name="esb", bufs=4) as e_pool, \
                 tc.tile_pool(name="bs", bufs=2) as bs_pool, \
                 tc.tile_pool(name="osb", bufs=4) as osb_pool, \
                 tc.tile_pool(name="misc", bufs=4) as misc_pool:

                for g in range(4):
                    # V projection for this group's 4 heads (token-major, 65th ones col)
                    wv_sb = wv_pool.tile([128, NCH, 256], F32R)
                    nc.sync.dma_start(
                        out=wv_sb,
                        in_=wqkv[:, 2 * D + 256 * g:2 * D + 256 * (g + 1)].rearrange("(c p) m -> p c m", p=128),
                    )
                    V_sb = vpool.tile([128, 16, 4, 65], F32R)
                    nc.sync.dma_start(
                        out=V_sb[:, :, :, 64],
                        in_=onesd[:, 0:64].rearrange("p (t g) -> p t g", t=16),
                    )
                    for t in range(16):
                        ps = pps_pool.tile([128, 256], F32, tag="pps")
                        for c in range(NCH):
                            nc.tensor.matmul(
                                out=ps, lhsT=_r(xT_sb[:, c, 128 * t:128 * (t + 1)]),
                                rhs=_r(wv_sb[:, c, :]),
                                start=(c == 0), stop=False,
                            )
                        nc.tensor.matmul(
                            out=ps, lhsT=_r(ones_sb),
                            rhs=_r(brow_sb[:, 256 * g:256 * (g + 1)]),
                            start=False, stop=True,
                        )
                        nc.vector.tensor_copy(
                            out=V_sb[:, t, :, 0:64],
                            in_=ps.rearrange("p (h d) -> p h d", h=4),
                        )

                    for lp in range(2):
                        pr = 2 * g + lp
                        # W-scaled per-block V column sums -> suffix sums
                        psb = pps_pool.tile([128, 4], F32, tag="pps")
                        for c in range(NCH):
                            nc.tensor.matmul(
                                out=psb, lhsT=_r(wv_sb[:, c, 128 * lp:128 * (lp + 1)]),
                                rhs=_r(xsum_sb[:, c, :]),
                                start=(c == 0), stop=(c == NCH - 1),
                            )
                        bs_sb = bs_pool.tile([128, 4], F32, tag="bs")
                        nc.vector.tensor_scalar(
                            out=bs_sb, in0=psb, scalar1=W_MASK,
                            scalar2=bv512_sb[:, pr:pr + 1], op0=ALU.mult, op1=ALU.add,
                        )
                        suf_sb = bs_pool.tile([128, 4], F32, tag="suf")
                        nc.vector.memset(suf_sb[:, 3:4], 0.0)
                        nc.vector.tensor_copy(out=suf_sb[:, 2:3], in_=bs_sb[:, 3:4])
                        nc.vector.tensor_add(out=suf_sb[:, 1:2], in0=bs_sb[:, 2:3], in1=suf_sb[:, 2:3])
                        nc.vector.tensor_add(out=suf_sb[:, 0:1], in0=bs_sb[:, 1:2], in1=suf_sb[:, 1:2])

                        # K projection for this pair (d-major)
                        wk_sb = wk_pool.tile([128, NCH, 128], F32R)
                        nc.sync.dma_start(
                            out=wk_sb,
                            in_=wqkv[:, D + 128 * pr:D + 128 * (pr + 1)].rearrange("(c p) m -> p c m", p=128),
                        )
                        KT_sb = kt_pool.tile([128, S], BF16)
                        for kg in range(4):
                            ps = pps_pool.tile([128, 512], F32, tag="pps")
                            for c in range(NCH):
                                nc.tensor.matmul(
                                    out=ps, lhsT=_r(wk_sb[:, c, :]),
                                    rhs=_r(xT_sb[:, c, 512 * kg:512 * (kg + 1)]),
                                    start=(c == 0), stop=(c == NCH - 1),
                                )
                            nc.vector.tensor_scalar_add(
                                out=KT_sb[:, 512 * kg:512 * (kg + 1)], in0=ps,
                                scalar1=b2h_sb[:, 8 + pr:9 + pr],
                            )

                        # attention, 2 heads interleaved per qtile to keep PE dense
                        for j in range(NJ):
                            for hl in range(2):
                                ghl = 2 * lp + hl
                                hsl = slice(64 * hl, 64 * (hl + 1))
                                # po cols 0:256 = PV accum + Z row; cols 256:512 = Z broadcast
                                po = ops_pool.tile([65, 512], F32, tag="ops")
                                for kb in range(j + 1):
                                    diag = kb == j
                                    pss = sps_pool.tile([128, 4, 256], F32)
                                    for s2 in range(4):
                                        nc.tensor.matmul(
                                            out=pss[:, s2, :],
                                            lhsT=_r(KT_sb[hsl, 512 * kb + 128 * s2:512 * kb + 128 * (s2 + 1)]),
                                            rhs=_r(QT_all[hsl, pr, 256 * j:256 * (j + 1)]),
                                            start=True, stop=True,
                                        )
                                    e_sb = e_pool.tile([128, 4, 256], F32R)
                                    nc.scalar.activation(out=e_sb, in_=pss, func=AF.Exp)
                                    if diag:
                                        ef = e_sb[:].rearrange("p a b -> p (a b)")
                                        nc.vector.tensor_mul(out=ef, in0=ef, in1=mmul_sb[:])
                                        nc.vector.tensor_add(out=ef, in0=ef, in1=madd_sb[:])
                                    for s2 in range(4):
                                        nc.tensor.matmul(
                                            out=po[:, 0:256],
                                            lhsT=_r(V_sb[:, 4 * kb + s2, ghl, :]),
                                            rhs=_r(e_sb[:, s2, :]),
                                            start=(kb == 0 and s2 == 0),
                                            stop=(kb == j and s2 == 3),
                                            skip_group_check=True,
                                        )
                                # epilogue: Z, broadcast, numerator correction, divide
                                nskip = S - KB * (j + 1)
                                zf = misc_pool.tile([1, 256], F32, tag="zf")
                                nc.vector.tensor_scalar_add(out=zf, in0=po[64:65, 0:256], scalar1=W_MASK * nskip)
                                zi = misc_pool.tile([1, 256], F32, tag="zi")
                                nc.vector.reciprocal_approx_fast(out=zi, in_=zf)
                                zr = misc_pool.tile([1, 256], F32R, tag="zr")
                                nc.vector.tensor_copy(out=zr, in_=zi)
                                nc.tensor.matmul(out=po[0:64, 256:512], lhsT=_r(ones_sb[:, 0:64]), rhs=_r(zr), start=True, stop=True)
                                nm = misc_pool.tile([64, 256], F32, tag="nm")
                                nc.vector.tensor_scalar_add(
                                    out=nm, in0=po[0:64, 0:256], scalar1=suf_sb[hsl, j:j + 1],
                                )
                                ot = osb_pool.tile([64, 256], F32R, tag="ot")
                                nc.vector.tensor_mul(out=ot, in0=nm, in1=po[0:64, 256:512])
                                nc.sync.dma_start(
                                    out=O_dr[128 * pr + 64 * hl:128 * pr + 64 * (hl + 1), 256 * j:256 * (j + 1)],
                                    in_=ot,
                                )

        # ---------- output projection ----------
        with tc.tile_pool(name="wosb", bufs=1) as wo_pool, \
             tc.tile_pool(name="ochunk", bufs=2) as oc_pool, \
             tc.tile_pool(name="fps", bufs=2, space="PSUM") as fps_pool, \
             tc.tile_pool(name="fout", bufs=3) as fo_pool:
            wo_sb = wo_pool.tile([128, NCH, 8, 128], F32R)
            nc.sync.dma_start(
                out=wo_sb,
                in_=wo[:].rearrange("(c p) (t m) -> p c t m", p=128, m=128),
            )
            for j in range(NJ):
                oj = oc_pool.tile([128, NCH, 256], F32R)
                nc.sync.dma_start(
                    out=oj,
                    in_=O_dr[:, 256 * j:256 * (j + 1)].rearrange("(c p) q -> p c q", p=128),
                )
                for dt_ in range(8):
                    ps = fps_pool.tile([128, 256], F32)
                    for c in range(NCH):
                        nc.tensor.matmul(
                            out=ps, lhsT=_r(wo_sb[:, c, dt_, :]), rhs=_r(oj[:, c, :]),
                            start=(c == 0), stop=(c == NCH - 1),
                        )
                    fo = fo_pool.tile([128, 256], F32)
                    nc.vector.tensor_scalar_add(out=fo, in0=ps, scalar1=bocol_sb[:, dt_:dt_ + 1])
                    nc.sync.dma_start(
                        out=outT[128 * dt_:128 * (dt_ + 1), 256 * j:256 * (j + 1)],
                        in_=fo,
                    )
    nc.compile()
    return nc


def qrows_for(p):
    return np.concatenate(
        [np.arange(QT * (2 * j + p), QT * (2 * j + p) + QT) for j in range(NJ)]
    )


def host_in_maps(x, Wqkv, bqkv, Wo, bo):
    x = np.ascontiguousarray(np.asarray(x, np.float32))
    Wqkv = np.ascontiguousarray(np.asarray(Wqkv, np.float32))
    bqkv = np.asarray(bqkv, np.float32)
    Wo = np.ascontiguousarray(np.asarray(Wo, np.float32))
    bo = np.asarray(bo, np.float32)

    b2h = np.ascontiguousarray(bqkv.reshape(24, 128).T)
    b2h[:, 0:8] /= 8.0
    brow = bqkv.reshape(1, 3 * D)
    bv512 = np.ascontiguousarray((W_MASK * 512.0 * bqkv[2 * D:].reshape(8, 128)).T)
    bocol = np.ascontiguousarray(bo.reshape(8, 128).T)
    onesd = np.ones((128, 128), np.float32)

    kap = np.arange(128)[:, None]
    r = np.arange(QT)[None, :]
    masks = {}
    for p in range(2):
        mm = np.zeros((128, 4, QT), np.float32)
        for s in range(4):
            mm[:, s, :] = (128 * s + kap <= QT * p + r)
        mm2 = np.ascontiguousarray(mm.reshape(128, 4 * QT))
        masks[p] = (mm2, np.ascontiguousarray((1.0 - mm2) * W_MASK))

    in_maps = []
    for core in range(8):
        b, p = core // 2, core % 2
        mma, mada = masks[p]
        in_maps.append({
            "xT": np.ascontiguousarray(x[b].T),
            "xqT": np.ascontiguousarray(x[b][qrows_for(p)].T),
            "wqkv": Wqkv,
            "wo": Wo,
            "b2h": b2h,
            "brow": brow,
            "bv512": bv512,
            "bocol": bocol,
            "onesd": onesd,
            "mmul": mma,
            "madd": mada,
        })
    return in_maps


_CACHED = {}


def get_program():
    if "nc" not in _CACHED:
        _CACHED["nc"] = build_program()
    return _CACHED["nc"]


def kernel(x, Wqkv, bqkv, Wo, bo):
    from concourse.bass_utils import run_bass_kernel_spmd

    nc = get_program()
    in_maps = host_in_maps(x, Wqkv, bqkv, Wo, bo)
    res = run_bass_kernel_spmd(nc, in_maps, core_ids=list(range(8)))
    out = np.zeros((B, S, D), np.float32)
    for core in range(8):
        b, p = core // 2, core % 2
        out[b, qrows_for(p), :] = res.results[core]["outT"].T
    return out



# revision 7
# speedup vs baseline: 1.0223x; 1.0223x over previous
"""Trainium2 Bass kernel for causal multi-head attention block.

Reference computation (B=4, S=2048, D=1024, H=16, HD=64, fp32):
    qkv = x @ Wqkv + bqkv; split q,k,v; per-head scaled scores;
    causal mask filled with -0.0001 (leaky, NOT -inf); softmax over all
    2048 keys; out = P @ V; out = out @ Wo + bo.

Sharding: 8 cores, core = (batch b = i//2, parity p = i%2). Each core
computes 1024 queries of its batch: query tiles t = 2j+p (j=0..3) of
256 queries. Causal block structure (512-key blocks per 256-query tile)
is identical on every core: qtile j needs key blocks 0..j -> one SPMD
program, zero cross-core communication.

v2 vs v1:
  - all matmul inputs bf16, host-prearranged into exact SBUF layouts
    (contiguous per-partition DMA lines, half the HBM bytes)
  - weights fully SBUF-resident up front -> no mid-loop PE DMA waits
  - PE warm-up matmuls during the initial x DMA (HAM stays at K=8/8)
  - diag-block leaky mask via one copy_predicated (e = M ? exp(S) : w)
  - head outputs kept in SBUF (no DRAM round trip before out-proj)
  - score matmuls for the 2 heads of a pair interleaved on 64-row
    groups for sub-array concurrency
"""

import math
from contextlib import ExitStack

import numpy as np

import concourse.bass as bass
import concourse.mybir as mybir
import concourse.tile as tile
from concourse import bacc

F32 = mybir.dt.float32
F32R = mybir.dt.float32r
BF16 = mybir.dt.bfloat16
U8 = mybir.dt.uint8
AF = mybir.ActivationFunctionType
ALU = mybir.AluOpType
AX = mybir.AxisListType

B, S, D, H, HD = 4, 2048, 1024, 16, 64
QL, QT, KB, NJ = 1024, 256, 512, 4    # queries/core, qtile, key block, n qtiles
NCH = D // 128                         # contraction chunks
PAIRS = H // 2
W_MASK = math.exp(-1e-4)


def build_program():
    nc = bacc.Bacc(
        "TRN2",
        target_bir_lowering=False,
        debug=False,
        num_devices=8,
    )
    xq = nc.declare_dram_parameter("xq", [128, NCH, QL], BF16, isOutput=False)
    xt = nc.declare_dram_parameter("xt", [128, NCH, S], BF16, isOutput=False)
    wq = nc.declare_dram_parameter("wq", [128, NCH, D], BF16, isOutput=False)
    wk = nc.declare_dram_parameter("wk", [128, NCH, D], BF16, isOutput=False)
    wv = nc.declare_dram_parameter("wv", [128, NCH, D], BF16, isOutput=False)
    wo = nc.declare_dram_parameter("wo", [128, NCH, D], BF16, isOutput=False)
    b2h = nc.declare_dram_parameter("b2h", [128, 16], F32, isOutput=False)
    brow = nc.declare_dram_parameter("brow", [1, D], BF16, isOutput=False)
    bv512 = nc.declare_dram_parameter("bv512", [128, 8], F32, isOutput=False)
    bocol = nc.declare_dram_parameter("bocol", [128, 8], F32, isOutput=False)
    maskp = nc.declare_dram_parameter("maskp", [128, 4 * QT], U8, isOutput=False)
    onesd = nc.declare_dram_parameter("onesd", [1, 64], F32R, isOutput=False)
    outT = nc.declare_dram_parameter("outT", [D, QL], F32, isOutput=True)

    with tile.TileContext(nc) as tc, ExitStack() as ctx, \
         nc.allow_low_precision(reason="bf16 compute, tolerance 2e-2"):
        consts = ctx.enter_context(tc.tile_pool(name="consts", bufs=1))
        # on-chip constants (no DMA): ones rows, masked-exp constant tile
        ones_bf = consts.tile([1, 128], BF16)
        nc.vector.memset(ones_bf, 1.0)
        ones_r = consts.tile([1, 64], F32R)
        nc.sync.dma_start(out=ones_r, in_=onesd[:])
        wtile = consts.tile([128, 512], BF16)
        nc.vector.memset(wtile, W_MASK)

        b2h_sb = consts.tile([128, 16], F32)
        nc.sync.dma_start(out=b2h_sb, in_=b2h[:])
        brow_sb = consts.tile([1, D], BF16)
        nc.sync.dma_start(out=brow_sb, in_=brow[:])
        bv512_sb = consts.tile([128, 8], F32)
        nc.sync.dma_start(out=bv512_sb, in_=bv512[:])
        bocol_sb = consts.tile([128, 8], F32)
        nc.sync.dma_start(out=bocol_sb, in_=bocol[:])
        maskp_sb = consts.tile([128, 4, QT], U8)
        nc.sync.dma_start(out=maskp_sb, in_=maskp[:].rearrange("p (a b) -> p a b", a=4))

        # resident weights + head-output buffer
        wq_sb = consts.tile([128, NCH, D], BF16)
        wk_sb = consts.tile([128, NCH, D], BF16)
        wv_sb = consts.tile([128, NCH, D], BF16)
        wo_sb = consts.tile([128, NCH, D], BF16)
        O_sb = consts.tile([128, NCH, QL], BF16)   # O^T: [hd, q], chunk c = pair
        QT_all = consts.tile([128, PAIRS, QL], BF16)

        with ExitStack() as ctx2:
            xt_pool = ctx2.enter_context(tc.tile_pool(name="xt", bufs=1))
            xt_sb = xt_pool.tile([128, NCH, S], BF16)

            with tc.tile_pool(name="xqp", bufs=1) as xq_pool, \
                 tc.tile_pool(name="qps", bufs=2, space="PSUM") as qps_pool:
                xq_sb = xq_pool.tile([128, NCH, QL], BF16)
                # DMA order = priority order: xq + wq feed the Q projection
                for c in range(NCH):
                    nc.sync.dma_start(out=xq_sb[:, c, :], in_=xq[:, c, :])
                    nc.sync.dma_start(out=wq_sb[:, c, :], in_=wq[:, c, :])
                for c in range(NCH):
                    nc.sync.dma_start(out=xt_sb[:, c, :], in_=xt[:, c, :])
                nc.sync.dma_start(out=wv_sb, in_=wv[:])
                nc.sync.dma_start(out=wk_sb, in_=wk[:])
                nc.sync.dma_start(out=wo_sb, in_=wo[:])

                # PE warm-up while the x DMA streams in (results unused)
                warm = qps_pool.tile([128, 512], F32, tag="warm")
                for _ in range(36):
                    nc.tensor.matmul(out=warm, lhsT=wtile[:, 0:128],
                                     rhs=wtile[:, 0:512], start=True, stop=True)

                # ---------- Q projection, all head pairs up front ----------
                for pr in range(PAIRS):
                    for g2 in range(2):
                        ps = qps_pool.tile([128, 512], F32, tag="q")
                        for c in range(NCH):
                            nc.tensor.matmul(
                                out=ps, lhsT=wq_sb[:, c, 128 * pr:128 * (pr + 1)],
                                rhs=xq_sb[:, c, 512 * g2:512 * (g2 + 1)],
                                start=(c == 0), stop=(c == NCH - 1),
                            )
                        # Wq pre-scaled by 1/8 on host; bias cols pre-divided
                        nc.vector.tensor_scalar_add(
                            out=QT_all[:, pr, 512 * g2:512 * (g2 + 1)], in0=ps,
                            scalar1=b2h_sb[:, pr:pr + 1],
                        )

            # per-512-block column sums of xt (for V block-sum corrections)
            xsum_sb = consts.tile([128, NCH, 4], BF16)
            for c in range(NCH):
                nc.vector.tensor_reduce(
                    out=xsum_sb[:, c, :],
                    in_=xt_sb[:, c, :].rearrange("p (b t) -> p b t", b=4),
                    axis=AX.X, op=ALU.add,
                )

            # ---------- main loop: 4 groups of 4 heads ----------
            vpool = ctx2.enter_context(tc.tile_pool(name="vsb", bufs=2))
            kt_pool = ctx2.enter_context(tc.tile_pool(name="kt", bufs=2))

            with tc.tile_pool(name="pps", bufs=2, space="PSUM") as pps_pool, \
                 tc.tile_pool(name="sps", bufs=2, space="PSUM") as sps_pool, \
                 tc.tile_pool(name="ops", bufs=1, space="PSUM") as ops_pool, \
                 tc.tile_pool(name="esb", bufs=3) as e_pool, \
                 tc.tile_pool(name="bs", bufs=2) as bs_pool, \
                 tc.tile_pool(name="misc", bufs=4) as misc_pool:

                for g in range(4):
                    # V projection for this group's 4 heads (token-major, 65th ones col)
                    V_sb = vpool.tile([128, 16, 4, 65], BF16)
                    nc.vector.memset(V_sb[:, :, :, 64], 1.0)
                    for t in range(16):
                        ps = pps_pool.tile([128, 256], F32, tag="pps")
                        for c in range(NCH):
                            nc.tensor.matmul(
                                out=ps, lhsT=xt_sb[:, c, 128 * t:128 * (t + 1)],
                                rhs=wv_sb[:, c, 256 * g:256 * (g + 1)],
                                start=(c == 0), stop=False,
                            )
                        nc.tensor.matmul(
                            out=ps, lhsT=ones_bf,
                            rhs=brow_sb[:, 256 * g:256 * (g + 1)],
                            start=False, stop=True,
                        )
                        nc.vector.tensor_copy(
                            out=V_sb[:, t, :, 0:64],
                            in_=ps.rearrange("p (h d) -> p h d", h=4),
                        )

                    for lp in range(2):
                        pr = 2 * g + lp
                        # W-scaled per-block V column sums -> suffix sums
                        psb = pps_pool.tile([128, 4], F32, tag="pps")
                        for c in range(NCH):
                            nc.tensor.matmul(
                                out=psb, lhsT=wv_sb[:, c, 256 * g + 128 * lp:256 * g + 128 * (lp + 1)],
                                rhs=xsum_sb[:, c, :],
                                start=(c == 0), stop=(c == NCH - 1),
                            )
                        bs_sb = bs_pool.tile([128, 4], F32, tag="bs")
                        nc.vector.tensor_scalar(
                            out=bs_sb, in0=psb, scalar1=W_MASK,
                            scalar2=bv512_sb[:, pr:pr + 1], op0=ALU.mult, op1=ALU.add,
                        )
                        suf_sb = bs_pool.tile([128, 4], F32, tag="suf")
                        nc.vector.memset(suf_sb[:, 3:4], 0.0)
                        nc.vector.tensor_copy(out=suf_sb[:, 2:3], in_=bs_sb[:, 3:4])
                        nc.vector.tensor_add(out=suf_sb[:, 1:2], in0=bs_sb[:, 2:3], in1=suf_sb[:, 2:3])
                        nc.vector.tensor_add(out=suf_sb[:, 0:1], in0=bs_sb[:, 1:2], in1=suf_sb[:, 1:2])

                        # K projection for this pair (d-major)
                        KT_sb = kt_pool.tile([128, S], BF16)
                        for kg in range(4):
                            ps = pps_pool.tile([128, 512], F32, tag="pps")
                            for c in range(NCH):
                                nc.tensor.matmul(
                                    out=ps, lhsT=wk_sb[:, c, 128 * pr:128 * (pr + 1)],
                                    rhs=xt_sb[:, c, 512 * kg:512 * (kg + 1)],
                                    start=(c == 0), stop=(c == NCH - 1),
                                )
                            nc.vector.tensor_scalar_add(
                                out=KT_sb[:, 512 * kg:512 * (kg + 1)], in0=ps,
                                scalar1=b2h_sb[:, 8 + pr:9 + pr],
                            )

                        # attention: per qtile, 2 heads on 64-row groups
                        for j in range(NJ):
                            po = [None, None]
                            for hl in range(2):
                                po[hl] = ops_pool.tile([65, 512], F32, tag=f"po{hl}", name=f"po{hl}")
                            for kb in range(j + 1):
                                diag = kb == j
                                for s2h in range(2):
                                    pss = [None, None]
                                    e_sb = [None, None]
                                    for hl in range(2):
                                        pss[hl] = sps_pool.tile([128, 2, 256], F32, tag=f"ss{hl}", name=f"ss{hl}")
                                    for s2 in range(2):
                                        for hl in range(2):
                                            hsl = slice(64 * hl, 64 * (hl + 1))
                                            k0 = 512 * kb + 128 * (2 * s2h + s2)
                                            nc.tensor.matmul(
                                                out=pss[hl][:, s2, :],
                                                lhsT=KT_sb[hsl, k0:k0 + 128],
                                                rhs=QT_all[hsl, pr, 256 * j:256 * (j + 1)],
                                                start=True, stop=True,
                                            )
                                    for hl in range(2):
                                        e_sb[hl] = e_pool.tile([128, 2, 256], BF16, tag=f"e{hl}", name=f"e{hl}")
                                        nc.scalar.activation(out=e_sb[hl], in_=pss[hl], func=AF.Exp)
                                        if diag:
                                            nc.vector.copy_predicated(
                                                out=e_sb[hl],
                                                mask=maskp_sb[:, 2 * s2h:2 * s2h + 2, :],
                                                data=wtile[:].rearrange("p (a b) -> p a b", a=2),
                                            )
                                    for s2 in range(2):
                                        for hl in range(2):
                                            nc.tensor.matmul(
                                                out=po[hl][:, 0:256],
                                                lhsT=V_sb[:, 4 * kb + 2 * s2h + s2, 2 * lp + hl, :],
                                                rhs=e_sb[hl][:, s2, :],
                                                start=(kb == 0 and s2h == 0 and s2 == 0),
                                                stop=(kb == j and s2h == 1 and s2 == 1),
                                                skip_group_check=True,
                                            )
                            # epilogue: Z, broadcast, numerator correction, divide
                            nskip = S - KB * (j + 1)
                            for hl in range(2):
                                hsl = slice(64 * hl, 64 * (hl + 1))
                                zf = misc_pool.tile([1, 256], F32, tag="zf")
                                nc.vector.tensor_scalar_add(out=zf, in0=po[hl][64:65, 0:256], scalar1=W_MASK * nskip)
                                zi = misc_pool.tile([1, 256], F32, tag="zi")
                                nc.vector.reciprocal_approx_fast(out=zi, in_=zf)
                                zr = misc_pool.tile([1, 256], F32R, tag="zr")
                                nc.vector.tensor_copy(out=zr, in_=zi)
                                nc.tensor.matmul(out=po[hl][0:64, 256:512], lhsT=ones_r, rhs=zr,
                                                 start=True, stop=True, skip_group_check=True)
                                nm = misc_pool.tile([64, 256], F32, tag="nm")
                                nc.vector.tensor_scalar_add(
                                    out=nm, in0=po[hl][0:64, 0:256],
                                    scalar1=suf_sb[hsl, j:j + 1],
                                )
                                nc.vector.tensor_mul(
                                    out=O_sb[hsl, pr, 256 * j:256 * (j + 1)],
                                    in0=nm, in1=po[hl][0:64, 256:512],
                                )

        # ---------- output projection ----------
        with tc.tile_pool(name="fps", bufs=2, space="PSUM") as fps_pool, \
             tc.tile_pool(name="fout", bufs=3) as fo_pool:
            for j in range(NJ):
                for dt_ in range(8):
                    ps = fps_pool.tile([128, 256], F32)
                    for c in range(NCH):
                        nc.tensor.matmul(
                            out=ps, lhsT=wo_sb[:, c, 128 * dt_:128 * (dt_ + 1)],
                            rhs=O_sb[:, c, 256 * j:256 * (j + 1)],
                            start=(c == 0), stop=(c == NCH - 1),
                        )
                    fo = fo_pool.tile([128, 256], F32)
                    nc.vector.tensor_scalar_add(out=fo, in0=ps, scalar1=bocol_sb[:, dt_:dt_ + 1])
                    nc.sync.dma_start(
                        out=outT[128 * dt_:128 * (dt_ + 1), 256 * j:256 * (j + 1)],
                        in_=fo,
                    )
    nc.compile()
    return nc


def qrows_for(p):
    return np.concatenate(
        [np.arange(QT * (2 * j + p), QT * (2 * j + p) + QT) for j in range(NJ)]
    )


def _bf16(a):
    import ml_dtypes
    return np.ascontiguousarray(a.astype(ml_dtypes.bfloat16))


def _chunked(mat2d, inner):
    """[D, inner] -> [128, NCH, inner] with row = 128*c + pdim."""
    return np.ascontiguousarray(mat2d.reshape(NCH, 128, inner).transpose(1, 0, 2))


def host_in_maps(x, Wqkv, bqkv, Wo, bo):
    x = np.asarray(x, np.float32)
    Wqkv = np.asarray(Wqkv, np.float32)
    bqkv = np.asarray(bqkv, np.float32)
    Wo = np.asarray(Wo, np.float32)
    bo = np.asarray(bo, np.float32)

    wq = _bf16(_chunked(Wqkv[:, 0:D] * 0.125, D))
    wk = _bf16(_chunked(Wqkv[:, D:2 * D], D))
    wv = _bf16(_chunked(Wqkv[:, 2 * D:3 * D], D))
    wo = _bf16(_chunked(Wo, D))

    b2h = np.empty((128, 16), np.float32)
    b2h[:, 0:8] = bqkv[0:D].reshape(8, 128).T / 8.0
    b2h[:, 8:16] = bqkv[D:2 * D].reshape(8, 128).T
    b2h = np.ascontiguousarray(b2h)
    brow = _bf16(bqkv[2 * D:].reshape(1, D))
    bv512 = np.ascontiguousarray((W_MASK * 512.0 * bqkv[2 * D:].reshape(8, 128)).T)
    bocol = np.ascontiguousarray(bo.reshape(8, 128).T)

    kap = np.arange(128)[:, None]
    r = np.arange(QT)[None, :]
    masks = {}
    for p in range(2):
        mm = np.zeros((128, 4, QT), np.float32)
        for s in range(4):
            mm[:, s, :] = (128 * s + kap > QT * p + r)   # 1 = masked
        masks[p] = np.ascontiguousarray(mm.reshape(128, 4 * QT).astype(np.uint8))

    in_maps = []
    for core in range(8):
        b, p = core // 2, core % 2
        in_maps.append({
            "xq": _bf16(x[b][qrows_for(p)].T.reshape(NCH, 128, QL).transpose(1, 0, 2)),
            "xt": _bf16(x[b].T.reshape(NCH, 128, S).transpose(1, 0, 2)),
            "wq": wq, "wk": wk, "wv": wv, "wo": wo,
            "b2h": b2h, "brow": brow, "bv512": bv512, "bocol": bocol,
            "maskp": masks[p],
            "onesd": np.ones((1, 64), np.float32),
        })
    return in_maps


_CACHED = {}


def get_program():
    if "nc" not in _CACHED:
        _CACHED["nc"] = build_program()
    return _CACHED["nc"]


def kernel(x, Wqkv, bqkv, Wo, bo):
    from concourse.bass_utils import run_bass_kernel_spmd

    nc = get_program()
    in_maps = host_in_maps(x, Wqkv, bqkv, Wo, bo)
    res = run_bass_kernel_spmd(nc, in_maps, core_ids=list(range(8)))
    out = np.zeros((B, S, D), np.float32)
    for core in range(8):
        b, p = core // 2, core % 2
        out[b, qrows_for(p), :] = res.results[core]["outT"].T
    return out


# revision 9
# speedup vs baseline: 1.0854x; 1.0618x over previous
"""Trainium2 Bass kernel for causal multi-head attention block.

Reference computation (B=4, S=2048, D=1024, H=16, HD=64, fp32):
    qkv = x @ Wqkv + bqkv; split q,k,v; per-head scaled scores;
    causal mask filled with -0.0001 (leaky, NOT -inf); softmax over all
    2048 keys; out = P @ V; out = out @ Wo + bo.

Sharding: 8 cores, core = (batch b = i//2, parity p = i%2). Each core
computes 1024 queries of its batch: query tiles t = 2j+p (j=0..3) of
256 queries. Causal block structure (512-key blocks per 256-query tile)
is identical on every core: qtile j needs key blocks 0..j -> one SPMD
program, zero cross-core communication.

v3: every matmul is N=512 where possible.  The PE weight-slot recycle
(LDW(i+2) waits on MM(i) completion, ~195 ns/MM floor for N=256 pairs)
makes narrow matmuls latency-bound; 512-wide moving operands are
stream-bound (~213 ns per MM).  Scores and PV process qtile PAIRS
(512 queries wide), V projection processes head-group pairs (512 V
columns wide), out-projection processes qtile pairs.  All matmul
inputs bf16 host-prearranged; weights SBUF-resident; leaky causal mask
via copy_predicated; head outputs stay in SBUF.
"""

import math
from contextlib import ExitStack

import numpy as np

import concourse.bass as bass
import concourse.mybir as mybir
import concourse.tile as tile
from concourse import bacc

F32 = mybir.dt.float32
F32R = mybir.dt.float32r
BF16 = mybir.dt.bfloat16
U8 = mybir.dt.uint8
AF = mybir.ActivationFunctionType
ALU = mybir.AluOpType
AX = mybir.AxisListType

B, S, D, H, HD = 4, 2048, 1024, 16, 64
QL, QT, KB, NJ = 1024, 256, 512, 4    # queries/core, qtile, key block, n qtiles
NCH = D // 128                         # contraction chunks
PAIRS = H // 2
W_MASK = math.exp(-1e-4)


def build_program():
    nc = bacc.Bacc(
        "TRN2",
        target_bir_lowering=False,
        debug=False,
        num_devices=8,
    )
    xq = nc.declare_dram_parameter("xq", [128, NCH, QL], BF16, isOutput=False)
    xt = nc.declare_dram_parameter("xt", [128, NCH, S], BF16, isOutput=False)
    wq = nc.declare_dram_parameter("wq", [128, NCH, D], BF16, isOutput=False)
    wk = nc.declare_dram_parameter("wk", [128, NCH, D], BF16, isOutput=False)
    wv = nc.declare_dram_parameter("wv", [128, NCH, D], BF16, isOutput=False)
    wo = nc.declare_dram_parameter("wo", [128, NCH, D], BF16, isOutput=False)
    b2h = nc.declare_dram_parameter("b2h", [128, 16], F32, isOutput=False)
    brow = nc.declare_dram_parameter("brow", [1, D], BF16, isOutput=False)
    bv512 = nc.declare_dram_parameter("bv512", [128, 8], F32, isOutput=False)
    bocol = nc.declare_dram_parameter("bocol", [128, 8], F32, isOutput=False)
    maskp = nc.declare_dram_parameter("maskp", [128, 4 * QT], U8, isOutput=False)
    onesd = nc.declare_dram_parameter("onesd", [1, 64], F32R, isOutput=False)
    outT = nc.declare_dram_parameter("outT", [D, QL], F32, isOutput=True)

    with tile.TileContext(nc) as tc, ExitStack() as ctx, \
         nc.allow_low_precision(reason="bf16 compute, tolerance 2e-2"):
        consts = ctx.enter_context(tc.tile_pool(name="consts", bufs=1))
        ones_bf = consts.tile([1, 128], BF16)
        nc.vector.memset(ones_bf, 1.0)
        ones_r = consts.tile([1, 64], F32R)
        nc.sync.dma_start(out=ones_r, in_=onesd[:])
        wtile = consts.tile([128, 512], BF16)
        nc.vector.memset(wtile, W_MASK)
        # W_MASK * nskip rows for the two qtiles of each J pair
        nskrow = consts.tile([1, 4, 256], F32, name="nskrow")
        for jj in range(4):
            nc.vector.memset(nskrow[:, jj, :], W_MASK * (S - KB * (jj + 1)))

        b2h_sb = consts.tile([128, 16], F32)
        nc.sync.dma_start(out=b2h_sb, in_=b2h[:])
        brow_sb = consts.tile([1, D], BF16)
        nc.sync.dma_start(out=brow_sb, in_=brow[:])
        bv512_sb = consts.tile([128, 8], F32)
        nc.sync.dma_start(out=bv512_sb, in_=bv512[:])
        bocol_sb = consts.tile([128, 8], F32)
        nc.sync.dma_start(out=bocol_sb, in_=bocol[:])
        maskp_sb = consts.tile([128, 4, QT], U8)
        nc.sync.dma_start(out=maskp_sb, in_=maskp[:].rearrange("p (a b) -> p a b", a=4))

        # resident weights + head-output buffer
        wq_sb = consts.tile([128, NCH, D], BF16)
        wk_sb = consts.tile([128, NCH, D], BF16)
        wv_sb = consts.tile([128, NCH, D], BF16)
        wo_sb = consts.tile([128, NCH, D], BF16)
        O_sb = consts.tile([128, NCH, QL], BF16)   # O^T: [hd, q], chunk c = pair
        QT_all = consts.tile([128, PAIRS, QL], BF16)

        with ExitStack() as ctx2:
            xt_pool = ctx2.enter_context(tc.tile_pool(name="xt", bufs=1))
            xt_sb = xt_pool.tile([128, NCH, S], BF16)

            with tc.tile_pool(name="xqp", bufs=1) as xq_pool, \
                 tc.tile_pool(name="qps", bufs=2, space="PSUM") as qps_pool:
                xq_sb = xq_pool.tile([128, NCH, QL], BF16)
                # DMA order = priority order: xq + wq feed the Q projection
                for c in range(NCH):
                    nc.sync.dma_start(out=xq_sb[:, c, :], in_=xq[:, c, :])
                    nc.sync.dma_start(out=wq_sb[:, c, :], in_=wq[:, c, :])
                for c in range(NCH):
                    nc.sync.dma_start(out=xt_sb[:, c, :], in_=xt[:, c, :])
                nc.sync.dma_start(out=wv_sb, in_=wv[:])
                nc.sync.dma_start(out=wk_sb, in_=wk[:])
                nc.sync.dma_start(out=wo_sb, in_=wo[:])

                # PE warm-up while the x DMA streams in (results unused)
                warm = qps_pool.tile([128, 512], F32, tag="warm")
                for _ in range(36):
                    nc.tensor.matmul(out=warm, lhsT=wtile[:, 0:128],
                                     rhs=wtile[:, 0:512], start=True, stop=True)

                # ---------- Q projection, all head pairs up front ----------
                for pr in range(PAIRS):
                    for g2 in range(2):
                        ps = qps_pool.tile([128, 512], F32, tag="q")
                        for c in range(NCH):
                            nc.tensor.matmul(
                                out=ps, lhsT=wq_sb[:, c, 128 * pr:128 * (pr + 1)],
                                rhs=xq_sb[:, c, 512 * g2:512 * (g2 + 1)],
                                start=(c == 0), stop=(c == NCH - 1),
                            )
                        # Wq pre-scaled by 1/8 on host; bias cols pre-divided
                        nc.vector.tensor_scalar_add(
                            out=QT_all[:, pr, 512 * g2:512 * (g2 + 1)], in0=ps,
                            scalar1=b2h_sb[:, pr:pr + 1],
                        )

            # per-512-block column sums of xt (for V block-sum corrections)
            xsum_sb = consts.tile([128, NCH, 4], BF16)
            for c in range(NCH):
                nc.vector.tensor_reduce(
                    out=xsum_sb[:, c, :],
                    in_=xt_sb[:, c, :].rearrange("p (b t) -> p b t", b=4),
                    axis=AX.X, op=ALU.add,
                )

            # ---------- main loop: 2 group-pairs of 8 heads ----------
            vpool = ctx2.enter_context(tc.tile_pool(name="vsb", bufs=2))
            kt_pool = ctx2.enter_context(tc.tile_pool(name="kt", bufs=2))

            with tc.tile_pool(name="pps", bufs=2, space="PSUM") as pps_pool, \
                 tc.tile_pool(name="sps", bufs=2, space="PSUM") as sps_pool, \
                 tc.tile_pool(name="ops", bufs=1, space="PSUM") as ops_pool, \
                 tc.tile_pool(name="esb", bufs=3) as e_pool, \
                 tc.tile_pool(name="bs", bufs=2) as bs_pool, \
                 tc.tile_pool(name="misc", bufs=2) as misc_pool:

                for gp in range(2):
                    # V projection for 8 heads (token-major, 65th ones col)
                    V_sb = vpool.tile([128, 16, 8, 65], BF16)
                    nc.vector.memset(V_sb[:, :, :, 64], 1.0)
                    for t in range(16):
                        ps = pps_pool.tile([128, 512], F32, tag="pps")
                        for c in range(NCH):
                            nc.tensor.matmul(
                                out=ps, lhsT=xt_sb[:, c, 128 * t:128 * (t + 1)],
                                rhs=wv_sb[:, c, 512 * gp:512 * (gp + 1)],
                                start=(c == 0), stop=False,
                            )
                        nc.tensor.matmul(
                            out=ps, lhsT=ones_bf,
                            rhs=brow_sb[:, 512 * gp:512 * (gp + 1)],
                            start=False, stop=True,
                        )
                        nc.vector.tensor_copy(
                            out=V_sb[:, t, :, 0:64],
                            in_=ps.rearrange("p (h d) -> p h d", h=8),
                        )

                    for lp in range(4):
                        pr = 4 * gp + lp
                        # W-scaled per-block V column sums -> suffix sums
                        psb = pps_pool.tile([128, 4], F32, tag="pps")
                        for c in range(NCH):
                            nc.tensor.matmul(
                                out=psb, lhsT=wv_sb[:, c, 128 * pr:128 * (pr + 1)],
                                rhs=xsum_sb[:, c, :],
                                start=(c == 0), stop=(c == NCH - 1),
                            )
                        bs_sb = bs_pool.tile([128, 4], F32, tag="bs")
                        nc.vector.tensor_scalar(
                            out=bs_sb, in0=psb, scalar1=W_MASK,
                            scalar2=bv512_sb[:, pr:pr + 1], op0=ALU.mult, op1=ALU.add,
                        )
                        suf_sb = bs_pool.tile([128, 4], F32, tag="suf")
                        nc.vector.memset(suf_sb[:, 3:4], 0.0)
                        nc.vector.tensor_copy(out=suf_sb[:, 2:3], in_=bs_sb[:, 3:4])
                        nc.vector.tensor_add(out=suf_sb[:, 1:2], in0=bs_sb[:, 2:3], in1=suf_sb[:, 2:3])
                        nc.vector.tensor_add(out=suf_sb[:, 0:1], in0=bs_sb[:, 1:2], in1=suf_sb[:, 1:2])

                        # K projection for this pair (d-major)
                        KT_sb = kt_pool.tile([128, S], BF16)
                        for kg in range(4):
                            ps = pps_pool.tile([128, 512], F32, tag="pps")
                            for c in range(NCH):
                                nc.tensor.matmul(
                                    out=ps, lhsT=wk_sb[:, c, 128 * pr:128 * (pr + 1)],
                                    rhs=xt_sb[:, c, 512 * kg:512 * (kg + 1)],
                                    start=(c == 0), stop=(c == NCH - 1),
                                )
                            nc.vector.tensor_scalar_add(
                                out=KT_sb[:, 512 * kg:512 * (kg + 1)], in0=ps,
                                scalar1=b2h_sb[:, 8 + pr:9 + pr],
                            )

                        # attention on qtile pairs (512 queries wide)
                        for J in range(2):
                            jlo, jhi = 2 * J, 2 * J + 1
                            po = [None, None]
                            for hl in range(2):
                                po[hl] = ops_pool.tile([65, 512], F32, tag=f"po{hl}", name=f"po{hl}")
                            for kb in range(jhi + 1):
                                last = kb == jhi        # N=256, qtile jhi only
                                dlo = kb == jlo         # jlo diag inside 512-wide tile
                                N = 256 if last else 512
                                qoff = 512 * J + (256 if last else 0)
                                for s2 in range(4):
                                    pss = [None, None]
                                    for hl in range(2):
                                        pss[hl] = sps_pool.tile([128, 512], F32, tag=f"ss{hl}", name=f"ss{hl}")
                                        hsl = slice(64 * hl, 64 * (hl + 1))
                                        k0 = 512 * kb + 128 * s2
                                        nc.tensor.matmul(
                                            out=pss[hl][:, 0:N],
                                            lhsT=KT_sb[hsl, k0:k0 + 128],
                                            rhs=QT_all[hsl, pr, qoff:qoff + N],
                                            start=True, stop=True,
                                        )
                                    for hl in range(2):
                                        e_sb = e_pool.tile([128, 512], BF16, tag=f"e{hl}", name=f"e{hl}")
                                        nc.scalar.activation(out=e_sb[:, 0:N], in_=pss[hl][:, 0:N], func=AF.Exp)
                                        if last or dlo:
                                            nc.vector.copy_predicated(
                                                out=e_sb[:, 0:256],
                                                mask=maskp_sb[:, s2, :],
                                                data=wtile[:, 0:256],
                                            )
                                        nc.tensor.matmul(
                                            out=po[hl][:, qoff - 512 * J:qoff - 512 * J + N],
                                            lhsT=V_sb[:, 4 * kb + s2, 2 * lp + hl, :],
                                            rhs=e_sb[:, 0:N],
                                            start=(kb == 0 and s2 == 0),
                                            stop=(kb == jhi and s2 == 3),
                                            skip_group_check=True,
                                        )
                            # epilogue: Z, broadcast, numerator correction, divide
                            for hl in range(2):
                                hsl = slice(64 * hl, 64 * (hl + 1))
                                zf = misc_pool.tile([1, 512], F32, tag="zf")
                                nc.vector.tensor_add(
                                    out=zf, in0=po[hl][64:65, 0:512],
                                    in1=nskrow[:, 2 * J:2 * J + 2, :].rearrange("o a b -> o (a b)"),
                                )
                                zi = misc_pool.tile([1, 512], F32, tag="zi")
                                nc.vector.reciprocal_approx_fast(out=zi, in_=zf)
                                zr = misc_pool.tile([1, 512], F32R, tag="zr")
                                nc.vector.tensor_copy(out=zr, in_=zi)
                                zbc = pps_pool.tile([64, 512], F32, tag="pps", name="zbc")
                                nc.tensor.matmul(out=zbc, lhsT=ones_r, rhs=zr,
                                                 start=True, stop=True)
                                nm = misc_pool.tile([64, 512], F32, tag="nm")
                                for half, jj in ((0, jlo), (1, jhi)):
                                    nc.vector.tensor_scalar_add(
                                        out=nm[:, 256 * half:256 * (half + 1)],
                                        in0=po[hl][0:64, 256 * half:256 * (half + 1)],
                                        scalar1=suf_sb[hsl, jj:jj + 1],
                                    )
                                nc.vector.tensor_mul(
                                    out=O_sb[hsl, pr, 512 * J:512 * (J + 1)],
                                    in0=nm, in1=zbc,
                                )

        # ---------- output projection ----------
        with tc.tile_pool(name="fps", bufs=2, space="PSUM") as fps_pool, \
             tc.tile_pool(name="fout", bufs=3) as fo_pool:
            for dt_ in range(8):
                for J in range(2):
                    ps = fps_pool.tile([128, 512], F32)
                    for c in range(NCH):
                        nc.tensor.matmul(
                            out=ps, lhsT=wo_sb[:, c, 128 * dt_:128 * (dt_ + 1)],
                            rhs=O_sb[:, c, 512 * J:512 * (J + 1)],
                            start=(c == 0), stop=(c == NCH - 1),
                        )
                    fo = fo_pool.tile([128, 512], F32)
                    nc.vector.tensor_scalar_add(out=fo, in0=ps, scalar1=bocol_sb[:, dt_:dt_ + 1])
                    nc.sync.dma_start(
                        out=outT[128 * dt_:128 * (dt_ + 1), 512 * J:512 * (J + 1)],
                        in_=fo,
                    )
    nc.compile()
    return nc


def qrows_for(p):
    return np.concatenate(
        [np.arange(QT * (2 * j + p), QT * (2 * j + p) + QT) for j in range(NJ)]
    )


def _bf16(a):
    import ml_dtypes
    return np.ascontiguousarray(a.astype(ml_dtypes.bfloat16))


def _chunked(mat2d, inner):
    """[D, inner] -> [128, NCH, inner] with row = 128*c + pdim."""
    return np.ascontiguousarray(mat2d.reshape(NCH, 128, inner).transpose(1, 0, 2))


def host_in_maps(x, Wqkv, bqkv, Wo, bo):
    x = np.asarray(x, np.float32)
    Wqkv = np.asarray(Wqkv, np.float32)
    bqkv = np.asarray(bqkv, np.float32)
    Wo = np.asarray(Wo, np.float32)
    bo = np.asarray(bo, np.float32)

    wq = _bf16(_chunked(Wqkv[:, 0:D] * 0.125, D))
    wk = _bf16(_chunked(Wqkv[:, D:2 * D], D))
    wv = _bf16(_chunked(Wqkv[:, 2 * D:3 * D], D))
    wo = _bf16(_chunked(Wo, D))

    b2h = np.empty((128, 16), np.float32)
    b2h[:, 0:8] = bqkv[0:D].reshape(8, 128).T / 8.0
    b2h[:, 8:16] = bqkv[D:2 * D].reshape(8, 128).T
    b2h = np.ascontiguousarray(b2h)
    brow = _bf16(bqkv[2 * D:].reshape(1, D))
    bv512 = np.ascontiguousarray((W_MASK * 512.0 * bqkv[2 * D:].reshape(8, 128)).T)
    bocol = np.ascontiguousarray(bo.reshape(8, 128).T)

    kap = np.arange(128)[:, None]
    r = np.arange(QT)[None, :]
    masks = {}
    for p in range(2):
        mm = np.zeros((128, 4, QT), np.float32)
        for s in range(4):
            mm[:, s, :] = (128 * s + kap > QT * p + r)   # 1 = masked
        masks[p] = np.ascontiguousarray(mm.reshape(128, 4 * QT).astype(np.uint8))

    in_maps = []
    for core in range(8):
        b, p = core // 2, core % 2
        in_maps.append({
            "xq": _bf16(x[b][qrows_for(p)].T.reshape(NCH, 128, QL).transpose(1, 0, 2)),
            "xt": _bf16(x[b].T.reshape(NCH, 128, S).transpose(1, 0, 2)),
            "wq": wq, "wk": wk, "wv": wv, "wo": wo,
            "b2h": b2h, "brow": brow, "bv512": bv512, "bocol": bocol,
            "maskp": masks[p],
            "onesd": np.ones((1, 64), np.float32),
        })
    return in_maps


_CACHED = {}


def get_program():
    if "nc" not in _CACHED:
        _CACHED["nc"] = build_program()
    return _CACHED["nc"]


def kernel(x, Wqkv, bqkv, Wo, bo):
    from concourse.bass_utils import run_bass_kernel_spmd

    nc = get_program()
    in_maps = host_in_maps(x, Wqkv, bqkv, Wo, bo)
    res = run_bass_kernel_spmd(nc, in_maps, core_ids=list(range(8)))
    out = np.zeros((B, S, D), np.float32)
    for core in range(8):
        b, p = core // 2, core % 2
        out[b, qrows_for(p), :] = res.results[core]["outT"].T
    return out


# revision 13
# speedup vs baseline: 1.1936x; 1.0996x over previous
"""Trainium2 Bass kernel for causal multi-head attention block.

Reference computation (B=4, S=2048, D=1024, H=16, HD=64, fp32):
    qkv = x @ Wqkv + bqkv; split q,k,v; per-head scaled scores;
    causal mask filled with -0.0001 (leaky, NOT -inf); softmax over all
    2048 keys; out = P @ V; out = out @ Wo + bo.

Sharding: 8 cores, core = (batch b = i//2, parity p = i%2). Each core
computes 1024 queries of its batch: query tiles t = 2j+p (j=0..3) of
256 queries; qtile j needs key blocks 0..j on every core -> one SPMD
program, zero cross-core communication.

v4 design notes:
  - All matmuls 512-wide moving operands where possible (the PE
    weight-slot recycle makes narrower MMs latency-bound).
  - Scores for both heads of a pair go to one [128,2,512] PSUM tile ->
    ONE exp activation per (kb,s2) unit ([128,1024]); ACT per-call
    overhead is 352 cycles so bigger calls matter.
  - The scalar engine (exp) and tensor engine run a tight
    producer/consumer loop in attention; projection matmuls for the
    NEXT pair are emitted interleaved into the attention stream (fill
    queue) so the PE never idles and HAM stays at K=8/8.
  - Elementwise bias/copy work moved to the idle GpSimd (Pool) engine;
    DVE keeps copy_predicated (mask), reciprocal, and the epilogue.
  - Leaky causal mask: e = mask ? w : exp(S) via one copy_predicated
    per diag unit (mask duplicated per head on host).
  - Z denominator via 65th all-ones V column; numerator correction for
    skipped key blocks via W-scaled suffix sums of per-block V sums.
"""

import math
from collections import deque
from contextlib import ExitStack

import numpy as np

import concourse.bass as bass
import concourse.mybir as mybir
import concourse.tile as tile
from concourse import bacc

F32 = mybir.dt.float32
F32R = mybir.dt.float32r
BF16 = mybir.dt.bfloat16
U8 = mybir.dt.uint8
AF = mybir.ActivationFunctionType
ALU = mybir.AluOpType
AX = mybir.AxisListType

B, S, D, H, HD = 4, 2048, 1024, 16, 64
QL, QT, KB, NJ = 1024, 256, 512, 4
NCH = D // 128
PAIRS = H // 2
W_MASK = math.exp(-1e-4)


def build_program():
    nc = bacc.Bacc(
        "TRN2",
        target_bir_lowering=False,
        debug=False,
        num_devices=8,
    )
    xq = nc.declare_dram_parameter("xq", [128, NCH, QL], BF16, isOutput=False)
    xt = nc.declare_dram_parameter("xt", [128, NCH, S], BF16, isOutput=False)
    wq = nc.declare_dram_parameter("wq", [128, NCH, D], BF16, isOutput=False)
    wk = nc.declare_dram_parameter("wk", [128, NCH, D], BF16, isOutput=False)
    wv = nc.declare_dram_parameter("wv", [128, NCH, D], BF16, isOutput=False)
    wo = nc.declare_dram_parameter("wo", [128, NCH, D], BF16, isOutput=False)
    b2h = nc.declare_dram_parameter("b2h", [128, 16], F32, isOutput=False)
    brow = nc.declare_dram_parameter("brow", [1, D], BF16, isOutput=False)
    bv512 = nc.declare_dram_parameter("bv512", [128, 8], F32, isOutput=False)
    bocol = nc.declare_dram_parameter("bocol", [128, 8], F32, isOutput=False)
    mdup = nc.declare_dram_parameter("mdup", [128, 8 * QT], U8, isOutput=False)
    onesd = nc.declare_dram_parameter("onesd", [1, 64], F32R, isOutput=False)
    outT = nc.declare_dram_parameter("outT", [D, QL], F32, isOutput=True)

    with tile.TileContext(nc) as tc, ExitStack() as ctx, \
         nc.allow_low_precision(reason="bf16 compute, tolerance 2e-2"):
        consts = ctx.enter_context(tc.tile_pool(name="consts", bufs=1))
        ones_bf = consts.tile([1, 128], BF16)
        nc.vector.memset(ones_bf, 1.0)
        ones_r = consts.tile([1, 64], F32R)
        nc.sync.dma_start(out=ones_r, in_=onesd[:])
        wtile = consts.tile([128, 512], BF16)
        nc.vector.memset(wtile, W_MASK)
        nskrow = consts.tile([1, 4, 256], F32, name="nskrow")
        for jj in range(4):
            nc.vector.memset(nskrow[:, jj, :], W_MASK * (S - KB * (jj + 1)))

        b2h_sb = consts.tile([128, 16], F32)
        nc.sync.dma_start(out=b2h_sb, in_=b2h[:])
        brow_sb = consts.tile([1, D], BF16)
        nc.sync.dma_start(out=brow_sb, in_=brow[:])
        bv512_sb = consts.tile([128, 8], F32)
        nc.sync.dma_start(out=bv512_sb, in_=bv512[:])
        bocol_sb = consts.tile([128, 8], F32)
        nc.sync.dma_start(out=bocol_sb, in_=bocol[:])
        mdup_sb = consts.tile([128, 4, 2, QT], U8)
        nc.sync.dma_start(out=mdup_sb, in_=mdup[:].rearrange("p (a h b) -> p a h b", a=4, h=2))

        wq_sb = consts.tile([128, NCH, D], BF16)
        wk_sb = consts.tile([128, NCH, D], BF16)
        wv_sb = consts.tile([128, NCH, D], BF16)
        wo_sb = consts.tile([128, NCH, D], BF16)
        O_sb = consts.tile([128, NCH, QL], BF16)
        QT_all = consts.tile([128, PAIRS, QL], BF16)
        xsum_sb = consts.tile([128, NCH, 4], BF16)

        with ExitStack() as ctx2:
            xt_pool = ctx2.enter_context(tc.tile_pool(name="xt", bufs=1))
            xq_pool = ctx2.enter_context(tc.tile_pool(name="xqp", bufs=1))
            vpool = ctx2.enter_context(tc.tile_pool(name="vsb", bufs=2))
            kt_pool = ctx2.enter_context(tc.tile_pool(name="kt", bufs=2))
            psum = ctx2.enter_context(tc.tile_pool(name="psum", bufs=1, space="PSUM"))
            e_pool = ctx2.enter_context(tc.tile_pool(name="esb", bufs=3))
            bs_pool = ctx2.enter_context(tc.tile_pool(name="bs", bufs=2))
            misc_pool = ctx2.enter_context(tc.tile_pool(name="misc", bufs=2))

            xt_sb = xt_pool.tile([128, NCH, S], BF16)
            xq_sb = xq_pool.tile([128, NCH, QL], BF16)

            # DMA order = priority order
            for c in range(NCH):
                nc.sync.dma_start(out=xq_sb[:, c, :], in_=xq[:, c, :])
                nc.sync.dma_start(out=wq_sb[:, c, :], in_=wq[:, c, :])
            for c in range(NCH):
                nc.sync.dma_start(out=xt_sb[:, c, :], in_=xt[:, c, :])
            nc.sync.dma_start(out=wv_sb, in_=wv[:])
            nc.sync.dma_start(out=wk_sb, in_=wk[:])
            nc.sync.dma_start(out=wo_sb, in_=wo[:])

            # PE warm-up while the x DMA streams in (results unused)
            warm = psum.tile([128, 512], F32, tag="pps", bufs=2, name="warm")
            for _ in range(36):
                nc.tensor.matmul(out=warm, lhsT=wtile[:, 0:128],
                                 rhs=wtile[:, 0:512], start=True, stop=True)

            # per-512-block column sums of xt (for V suffix corrections)
            for c in range(NCH):
                nc.vector.tensor_reduce(
                    out=xsum_sb[:, c, :],
                    in_=xt_sb[:, c, :].rearrange("p (b t) -> p b t", b=4),
                    axis=AX.X, op=ALU.add,
                )

            # ---------------- fill-queue machinery ----------------
            fill = deque()
            kt_of = {}
            suf_of = {}
            v_of = {}

            def drain(n_mm):
                while n_mm > 0 and fill:
                    n_mm -= fill.popleft()()

            def flush():
                while fill:
                    fill.popleft()()

            def push_qproj(pr):
                def mk(g2):
                    def go():
                        ps = psum.tile([128, 512], F32, tag="pps", bufs=2, name="qp")
                        for c in range(NCH):
                            nc.tensor.matmul(
                                out=ps, lhsT=wq_sb[:, c, 128 * pr:128 * (pr + 1)],
                                rhs=xq_sb[:, c, 512 * g2:512 * (g2 + 1)],
                                start=(c == 0), stop=(c == NCH - 1),
                            )
                        nc.vector.tensor_scalar_add(
                            out=QT_all[:, pr, 512 * g2:512 * (g2 + 1)], in0=ps,
                            scalar1=b2h_sb[:, pr:pr + 1],
                        )
                        return NCH
                    return go
                for g2 in range(2):
                    fill.append(mk(g2))

            def push_kproj(pr):
                KT_sb = kt_pool.tile([128, S], BF16, name="KT")
                kt_of[pr] = KT_sb

                def mk(kg):
                    def go():
                        ps = psum.tile([128, 512], F32, tag="pps", bufs=2, name="kp")
                        for c in range(NCH):
                            nc.tensor.matmul(
                                out=ps, lhsT=wk_sb[:, c, 128 * pr:128 * (pr + 1)],
                                rhs=xt_sb[:, c, 512 * kg:512 * (kg + 1)],
                                start=(c == 0), stop=(c == NCH - 1),
                            )
                        nc.vector.tensor_scalar_add(
                            out=KT_sb[:, 512 * kg:512 * (kg + 1)], in0=ps,
                            scalar1=b2h_sb[:, 8 + pr:9 + pr],
                        )
                        return NCH
                    return go
                for kg in range(4):
                    fill.append(mk(kg))

            def push_psb(pr):
                def go():
                    psb = psum.tile([128, 4], F32, tag="pps", bufs=2, name="psb")
                    for c in range(NCH):
                        nc.tensor.matmul(
                            out=psb, lhsT=wv_sb[:, c, 128 * pr:128 * (pr + 1)],
                            rhs=xsum_sb[:, c, :],
                            start=(c == 0), stop=(c == NCH - 1),
                        )
                    bs_sb = bs_pool.tile([128, 4], F32, tag="bs", name="bs_sb")
                    nc.vector.tensor_scalar(
                        out=bs_sb, in0=psb, scalar1=W_MASK,
                        scalar2=bv512_sb[:, pr:pr + 1], op0=ALU.mult, op1=ALU.add,
                    )
                    suf_sb = bs_pool.tile([128, 4], F32, tag="suf", name="suf_sb")
                    suf_of[pr] = suf_sb
                    nc.vector.memset(suf_sb[:, 3:4], 0.0)
                    nc.vector.tensor_copy(out=suf_sb[:, 2:3], in_=bs_sb[:, 3:4])
                    nc.vector.tensor_add(out=suf_sb[:, 1:2], in0=bs_sb[:, 2:3], in1=suf_sb[:, 2:3])
                    nc.vector.tensor_add(out=suf_sb[:, 0:1], in0=bs_sb[:, 1:2], in1=suf_sb[:, 1:2])
                    return NCH
                fill.append(go)

            def push_vproj(gp):
                V_sb = vpool.tile([128, 16, 8, 65], BF16, name="V_sb")
                v_of[gp] = V_sb

                def ones_go():
                    nc.vector.memset(V_sb[:, :, :, 64], 1.0)
                    return 0
                fill.append(ones_go)

                def mk(t):
                    def go():
                        ps = psum.tile([128, 512], F32, tag="pps", bufs=2, name="vp")
                        for c in range(NCH):
                            nc.tensor.matmul(
                                out=ps, lhsT=xt_sb[:, c, 128 * t:128 * (t + 1)],
                                rhs=wv_sb[:, c, 512 * gp:512 * (gp + 1)],
                                start=(c == 0), stop=False,
                            )
                        nc.tensor.matmul(
                            out=ps, lhsT=ones_bf,
                            rhs=brow_sb[:, 512 * gp:512 * (gp + 1)],
                            start=False, stop=True,
                        )
                        nc.vector.tensor_copy(
                            out=V_sb[:, t, :, 0:64],
                            in_=ps.rearrange("p (h d) -> p h d", h=8),
                        )
                        return NCH + 1
                    return go
                for t in range(16):
                    fill.append(mk(t))

            # ---------------- bootstrap: pair 0 (+1) prereqs ----------------
            push_qproj(0)
            push_psb(0)
            push_vproj(0)
            push_kproj(0)
            flush()
            push_qproj(1)
            push_kproj(1)
            push_psb(1)

            # ---------------- main attention loop ----------------
            for pr in range(PAIRS):
                gp, lpi = pr // 4, pr % 4
                KT_sb, suf_sb, V_sb = kt_of[pr], suf_of[pr], v_of[gp]
                for J in range(2):
                    jlo, jhi = 2 * J, 2 * J + 1
                    po = [None, None]
                    for hl in range(2):
                        po[hl] = psum.tile([65, 512], F32, tag=f"po{hl}", bufs=1, name=f"po{hl}")
                    for kb in range(jhi + 1):
                        last = kb == jhi
                        dlo = kb == jlo
                        N = 256 if last else 512
                        qoff = 512 * J + (256 if last else 0)
                        for s2 in range(4):
                            pss = psum.tile([128, 2, 512], F32, tag="ss", bufs=2, name="ss")
                            k0 = 512 * kb + 128 * s2
                            for hl in range(2):
                                hsl = slice(64 * hl, 64 * (hl + 1))
                                nc.tensor.matmul(
                                    out=pss[:, hl, 0:N],
                                    lhsT=KT_sb[hsl, k0:k0 + 128],
                                    rhs=QT_all[hsl, pr, qoff:qoff + N],
                                    start=True, stop=True,
                                )
                            e_sb = e_pool.tile([128, 2, 512], BF16, tag="e", name="e_sb")
                            nc.scalar.activation(out=e_sb[:, :, 0:N], in_=pss[:, :, 0:N], func=AF.Exp)
                            if last or dlo:
                                nc.vector.copy_predicated(
                                    out=e_sb[:, :, 0:256],
                                    mask=mdup_sb[:, s2, :, :],
                                    data=wtile[:].rearrange("p (h b) -> p h b", h=2),
                                )
                            for hl in range(2):
                                nc.tensor.matmul(
                                    out=po[hl][:, qoff - 512 * J:qoff - 512 * J + N],
                                    lhsT=V_sb[:, 4 * kb + s2, 2 * lpi + hl, :],
                                    rhs=e_sb[:, hl, 0:N],
                                    start=(kb == 0 and s2 == 0),
                                    stop=(kb == jhi and s2 == 3),
                                    skip_group_check=True,
                                )
                            drain(2)
                    # epilogue: Z, broadcast, numerator correction, divide
                    for hl in range(2):
                        hsl = slice(64 * hl, 64 * (hl + 1))
                        zfs = e_pool.tile([1, 512], F32R, tag="e", name="zfs")
                        nc.vector.tensor_add(
                            out=zfs, in0=po[hl][64:65, 0:512],
                            in1=nskrow[:, 2 * J:2 * J + 2, :].rearrange("o a b -> o (a b)"),
                        )
                        zbc = psum.tile([64, 512], F32, tag="pps", bufs=2, name="zbc")
                        nc.tensor.matmul(out=zbc, lhsT=ones_r, rhs=zfs,
                                         start=True, stop=True)
                        rzb = misc_pool.tile([64, 512], F32, tag="rzb")
                        nc.vector.reciprocal_approx_fast(out=rzb, in_=zbc)
                        nm = e_pool.tile([64, 512], F32, tag="e", name="nm")
                        for half, jj in ((0, jlo), (1, jhi)):
                            nc.vector.tensor_scalar_add(
                                out=nm[:, 256 * half:256 * (half + 1)],
                                in0=po[hl][0:64, 256 * half:256 * (half + 1)],
                                scalar1=suf_sb[hsl, jj:jj + 1],
                            )
                        nc.gpsimd.tensor_mul(
                            out=O_sb[hsl, pr, 512 * J:512 * (J + 1)],
                            in0=nm, in1=rzb,
                        )
                flush()
                if pr + 2 < PAIRS:
                    push_qproj(pr + 2)
                    push_kproj(pr + 2)
                    push_psb(pr + 2)
                if pr == 1:
                    push_vproj(1)

            # ---------------- output projection ----------------
            for dt_ in range(8):
                for J in range(2):
                    ps = psum.tile([128, 512], F32, tag="pps", bufs=2, name="fps")
                    for c in range(NCH):
                        nc.tensor.matmul(
                            out=ps, lhsT=wo_sb[:, c, 128 * dt_:128 * (dt_ + 1)],
                            rhs=O_sb[:, c, 512 * J:512 * (J + 1)],
                            start=(c == 0), stop=(c == NCH - 1),
                        )
                    fo = e_pool.tile([128, 512], F32, tag="e", name="fo")
                    nc.vector.tensor_scalar_add(out=fo, in0=ps, scalar1=bocol_sb[:, dt_:dt_ + 1])
                    nc.sync.dma_start(
                        out=outT[128 * dt_:128 * (dt_ + 1), 512 * J:512 * (J + 1)],
                        in_=fo,
                    )
    nc.compile()
    return nc


def qrows_for(p):
    return np.concatenate(
        [np.arange(QT * (2 * j + p), QT * (2 * j + p) + QT) for j in range(NJ)]
    )


def _bf16(a):
    import ml_dtypes
    return np.ascontiguousarray(a.astype(ml_dtypes.bfloat16))


def _chunked(mat2d, inner):
    return np.ascontiguousarray(mat2d.reshape(NCH, 128, inner).transpose(1, 0, 2))


def host_in_maps(x, Wqkv, bqkv, Wo, bo):
    x = np.asarray(x, np.float32)
    Wqkv = np.asarray(Wqkv, np.float32)
    bqkv = np.asarray(bqkv, np.float32)
    Wo = np.asarray(Wo, np.float32)
    bo = np.asarray(bo, np.float32)

    wq = _bf16(_chunked(Wqkv[:, 0:D] * 0.125, D))
    wk = _bf16(_chunked(Wqkv[:, D:2 * D], D))
    wv = _bf16(_chunked(Wqkv[:, 2 * D:3 * D], D))
    wo = _bf16(_chunked(Wo, D))

    b2h = np.empty((128, 16), np.float32)
    b2h[:, 0:8] = bqkv[0:D].reshape(8, 128).T / 8.0
    b2h[:, 8:16] = bqkv[D:2 * D].reshape(8, 128).T
    b2h = np.ascontiguousarray(b2h)
    brow = _bf16(bqkv[2 * D:].reshape(1, D))
    bv512 = np.ascontiguousarray((W_MASK * 512.0 * bqkv[2 * D:].reshape(8, 128)).T)
    bocol = np.ascontiguousarray(bo.reshape(8, 128).T)

    kap = np.arange(128)[:, None]
    r = np.arange(QT)[None, :]
    masks = {}
    for p in range(2):
        mm = np.zeros((128, 4, 1, QT), np.uint8)
        for s in range(4):
            mm[:, s, 0, :] = (128 * s + kap > QT * p + r)   # 1 = masked
        md = np.repeat(mm, 2, axis=2)                        # dup per head
        masks[p] = np.ascontiguousarray(md.reshape(128, 8 * QT))

    in_maps = []
    for core in range(8):
        b, p = core // 2, core % 2
        in_maps.append({
            "xq": _bf16(x[b][qrows_for(p)].T.reshape(NCH, 128, QL).transpose(1, 0, 2)),
            "xt": _bf16(x[b].T.reshape(NCH, 128, S).transpose(1, 0, 2)),
            "wq": wq, "wk": wk, "wv": wv, "wo": wo,
            "b2h": b2h, "brow": brow, "bv512": bv512, "bocol": bocol,
            "mdup": masks[p],
            "onesd": np.ones((1, 64), np.float32),
        })
    return in_maps


_CACHED = {}


def get_program():
    if "nc" not in _CACHED:
        _CACHED["nc"] = build_program()
    return _CACHED["nc"]


def kernel(x, Wqkv, bqkv, Wo, bo):
    from concourse.bass_utils import run_bass_kernel_spmd

    nc = get_program()
    in_maps = host_in_maps(x, Wqkv, bqkv, Wo, bo)
    res = run_bass_kernel_spmd(nc, in_maps, core_ids=list(range(8)))
    out = np.zeros((B, S, D), np.float32)
    for core in range(8):
        b, p = core // 2, core % 2
        out[b, qrows_for(p), :] = res.results[core]["outT"].T
    return out


# revision 15
# speedup vs baseline: 1.3790x; 1.1554x over previous
"""Trainium2 Bass kernel for causal multi-head attention block.

Reference computation (B=4, S=2048, D=1024, H=16, HD=64, fp32):
    qkv = x @ Wqkv + bqkv; split q,k,v; per-head scaled scores;
    causal mask filled with -0.0001 (leaky, NOT -inf); softmax over all
    2048 keys; out = P @ V; out = out @ Wo + bo.

Sharding: 8 cores, core = (batch b = i//2, parity p = i%2). Each core
computes 1024 queries of its batch: query tiles t = 2j+p (j=0..3) of
256 queries; qtile j needs key blocks 0..j on every core -> one SPMD
program, zero cross-core communication.

v4 design notes:
  - All matmuls 512-wide moving operands where possible (the PE
    weight-slot recycle makes narrower MMs latency-bound).
  - Scores for both heads of a pair go to one [128,2,512] PSUM tile ->
    ONE exp activation per (kb,s2) unit ([128,1024]); ACT per-call
    overhead is 352 cycles so bigger calls matter.
  - The scalar engine (exp) and tensor engine run a tight
    producer/consumer loop in attention; projection matmuls for the
    NEXT pair are emitted interleaved into the attention stream (fill
    queue) so the PE never idles and HAM stays at K=8/8.
  - Elementwise bias/copy work moved to the idle GpSimd (Pool) engine;
    DVE keeps copy_predicated (mask), reciprocal, and the epilogue.
  - Leaky causal mask: e = mask ? w : exp(S) via one copy_predicated
    per diag unit (mask duplicated per head on host).
  - Z denominator via 65th all-ones V column; numerator correction for
    skipped key blocks via W-scaled suffix sums of per-block V sums.
"""

import math
from collections import deque
from contextlib import ExitStack

import numpy as np

import concourse.bass as bass
import concourse.mybir as mybir
import concourse.tile as tile
from concourse import bacc

F32 = mybir.dt.float32
F32R = mybir.dt.float32r
BF16 = mybir.dt.bfloat16
U8 = mybir.dt.uint8
AF = mybir.ActivationFunctionType
ALU = mybir.AluOpType
AX = mybir.AxisListType

B, S, D, H, HD = 4, 2048, 1024, 16, 64
QL, QT, KB, NJ = 1024, 256, 512, 4
NCH = D // 128
PAIRS = H // 2
W_MASK = math.exp(-1e-4)


def build_program():
    nc = bacc.Bacc(
        "TRN2",
        target_bir_lowering=False,
        debug=False,
        num_devices=8,
    )
    xq = nc.declare_dram_parameter("xq", [128, NCH, QL], BF16, isOutput=False)
    xt = nc.declare_dram_parameter("xt", [128, NCH, S], BF16, isOutput=False)
    wq = nc.declare_dram_parameter("wq", [128, NCH, D], BF16, isOutput=False)
    wk = nc.declare_dram_parameter("wk", [128, NCH, D], BF16, isOutput=False)
    wv = nc.declare_dram_parameter("wv", [128, NCH, D], BF16, isOutput=False)
    wo = nc.declare_dram_parameter("wo", [128, NCH, D], BF16, isOutput=False)
    b2h = nc.declare_dram_parameter("b2h", [128, 16], F32, isOutput=False)
    brow = nc.declare_dram_parameter("brow", [1, D], BF16, isOutput=False)
    bv512 = nc.declare_dram_parameter("bv512", [128, 8], F32, isOutput=False)
    bocol = nc.declare_dram_parameter("bocol", [128, 8], F32, isOutput=False)
    mdup = nc.declare_dram_parameter("mdup", [128, 8 * QT], U8, isOutput=False)
    onesd = nc.declare_dram_parameter("onesd", [1, 64], F32R, isOutput=False)
    outT = nc.declare_dram_parameter("outT", [D, QL], F32, isOutput=True)

    with tile.TileContext(nc) as tc, ExitStack() as ctx, \
         nc.allow_low_precision(reason="bf16 compute, tolerance 2e-2"):
        consts = ctx.enter_context(tc.tile_pool(name="consts", bufs=1))
        ones_bf = consts.tile([1, 128], BF16)
        nc.vector.memset(ones_bf, 1.0)
        ones_r = consts.tile([1, 64], F32R)
        nc.sync.dma_start(out=ones_r, in_=onesd[:])
        wtile = consts.tile([128, 512], BF16)
        nc.vector.memset(wtile, W_MASK)
        nskrow = consts.tile([1, 4, 256], F32, name="nskrow")
        for jj in range(4):
            nc.vector.memset(nskrow[:, jj, :], W_MASK * (S - KB * (jj + 1)))

        b2h_sb = consts.tile([128, 16], F32)
        nc.sync.dma_start(out=b2h_sb, in_=b2h[:])
        brow_sb = consts.tile([1, D], BF16)
        nc.sync.dma_start(out=brow_sb, in_=brow[:])
        bv512_sb = consts.tile([128, 8], F32)
        nc.sync.dma_start(out=bv512_sb, in_=bv512[:])
        bocol_sb = consts.tile([128, 8], F32)
        nc.sync.dma_start(out=bocol_sb, in_=bocol[:])
        mdup_sb = consts.tile([128, 4, 2, QT], U8)
        nc.sync.dma_start(out=mdup_sb, in_=mdup[:].rearrange("p (a h b) -> p a h b", a=4, h=2))

        wk_sb = consts.tile([128, NCH, D], BF16)
        wv_sb = consts.tile([128, NCH, D], BF16)
        wo_sb = consts.tile([128, NCH, D], BF16)
        O_sb = consts.tile([128, NCH, QL], BF16)
        QT_all = consts.tile([128, PAIRS, QL], BF16)
        xsum_sb = consts.tile([128, NCH, 4], BF16)

        with ExitStack() as ctx2:
            xt_pool = ctx2.enter_context(tc.tile_pool(name="xt", bufs=1))
            psum = ctx2.enter_context(tc.tile_pool(name="psum", bufs=1, space="PSUM"))

            xt_sb = xt_pool.tile([128, NCH, S], BF16)

            with tc.tile_pool(name="xqp", bufs=1) as xq_pool:
                xq_sb = xq_pool.tile([128, NCH, QL], BF16)
                wq_sb = xq_pool.tile([128, NCH, D], BF16)
                # DMA order = priority order
                for c in range(NCH):
                    nc.sync.dma_start(out=xq_sb[:, c, :], in_=xq[:, c, :])
                    nc.sync.dma_start(out=wq_sb[:, c, :], in_=wq[:, c, :])
                for c in range(NCH):
                    nc.sync.dma_start(out=xt_sb[:, c, :], in_=xt[:, c, :])
                nc.sync.dma_start(out=wk_sb, in_=wk[:])
                nc.sync.dma_start(out=wv_sb, in_=wv[:])
                nc.sync.dma_start(out=wo_sb, in_=wo[:])

                # PE warm-up while the x DMA streams in (results unused)
                warm = psum.tile([128, 512], F32, tag="pps", bufs=2, name="warm")
                for _ in range(36):
                    nc.tensor.matmul(out=warm, lhsT=wtile[:, 0:128],
                                     rhs=wtile[:, 0:512], start=True, stop=True)

                # Q projection for ALL pairs up front (only needs xq+wq)
                for pr in range(PAIRS):
                    for g2 in range(2):
                        ps = psum.tile([128, 512], F32, tag="pps", bufs=2, name="qp")
                        for c in range(NCH):
                            nc.tensor.matmul(
                                out=ps, lhsT=wq_sb[:, c, 128 * pr:128 * (pr + 1)],
                                rhs=xq_sb[:, c, 512 * g2:512 * (g2 + 1)],
                                start=(c == 0), stop=(c == NCH - 1),
                            )
                        nc.vector.tensor_scalar_add(
                            out=QT_all[:, pr, 512 * g2:512 * (g2 + 1)], in0=ps,
                            scalar1=b2h_sb[:, pr:pr + 1],
                        )

            vpool = ctx2.enter_context(tc.tile_pool(name="vsb", bufs=2))
            kt_pool = ctx2.enter_context(tc.tile_pool(name="kt", bufs=2))
            e_pool = ctx2.enter_context(tc.tile_pool(name="esb", bufs=4))
            bs_pool = ctx2.enter_context(tc.tile_pool(name="bs", bufs=2))
            misc_pool = ctx2.enter_context(tc.tile_pool(name="misc", bufs=2))

            # ---------------- fill-queue machinery ----------------
            fill = deque()
            kt_of = {}
            suf_of = {}
            v_of = {}

            def drain(n_mm):
                while n_mm > 0 and fill:
                    n_mm -= fill.popleft()()

            def flush():
                while fill:
                    fill.popleft()()

            def push_kproj(pr):
                KT_sb = kt_pool.tile([128, S], BF16, name="KT")
                kt_of[pr] = KT_sb

                def mk(kg):
                    def go():
                        ps = psum.tile([128, 512], F32, tag="pps", bufs=2, name="kp")
                        for c in range(NCH):
                            nc.tensor.matmul(
                                out=ps, lhsT=wk_sb[:, c, 128 * pr:128 * (pr + 1)],
                                rhs=xt_sb[:, c, 512 * kg:512 * (kg + 1)],
                                start=(c == 0), stop=(c == NCH - 1),
                            )
                        nc.vector.tensor_scalar_add(
                            out=KT_sb[:, 512 * kg:512 * (kg + 1)], in0=ps,
                            scalar1=b2h_sb[:, 8 + pr:9 + pr],
                        )
                        return NCH
                    return go
                for kg in range(4):
                    fill.append(mk(kg))

            def push_psb(pr):
                def go():
                    psb = psum.tile([128, 4], F32, tag="pps", bufs=2, name="psb")
                    for c in range(NCH):
                        nc.tensor.matmul(
                            out=psb, lhsT=wv_sb[:, c, 128 * pr:128 * (pr + 1)],
                            rhs=xsum_sb[:, c, :],
                            start=(c == 0), stop=(c == NCH - 1),
                        )
                    bs_sb = bs_pool.tile([128, 4], F32, tag="bs", name="bs_sb")
                    nc.vector.tensor_scalar(
                        out=bs_sb, in0=psb, scalar1=W_MASK,
                        scalar2=bv512_sb[:, pr:pr + 1], op0=ALU.mult, op1=ALU.add,
                    )
                    suf_sb = bs_pool.tile([128, 4], F32, tag="suf", name="suf_sb")
                    suf_of[pr] = suf_sb
                    nc.vector.memset(suf_sb[:, 3:4], 0.0)
                    nc.vector.tensor_copy(out=suf_sb[:, 2:3], in_=bs_sb[:, 3:4])
                    nc.vector.tensor_add(out=suf_sb[:, 1:2], in0=bs_sb[:, 2:3], in1=suf_sb[:, 2:3])
                    nc.vector.tensor_add(out=suf_sb[:, 0:1], in0=bs_sb[:, 1:2], in1=suf_sb[:, 1:2])
                    return NCH
                fill.append(go)

            def push_vproj(gp):
                V_sb = vpool.tile([128, 16, 8, 65], BF16, name="V_sb")
                v_of[gp] = V_sb

                def ones_go():
                    nc.vector.memset(V_sb[:, :, :, 64], 1.0)
                    return 0
                fill.append(ones_go)

                def mk(t):
                    def go():
                        ps = psum.tile([128, 512], F32, tag="pps", bufs=2, name="vp")
                        for c in range(NCH):
                            nc.tensor.matmul(
                                out=ps, lhsT=xt_sb[:, c, 128 * t:128 * (t + 1)],
                                rhs=wv_sb[:, c, 512 * gp:512 * (gp + 1)],
                                start=(c == 0), stop=False,
                            )
                        nc.tensor.matmul(
                            out=ps, lhsT=ones_bf,
                            rhs=brow_sb[:, 512 * gp:512 * (gp + 1)],
                            start=False, stop=True,
                        )
                        nc.vector.tensor_copy(
                            out=V_sb[:, t, :, 0:64],
                            in_=ps.rearrange("p (h d) -> p h d", h=8),
                        )
                        return NCH + 1
                    return go
                for t in range(16):
                    fill.append(mk(t))

            # ---------------- bootstrap: pair 0 (+1) prereqs ----------------
            push_kproj(0)
            flush()
            # xsum on DVE after the K TS ops (keeps Q/K epilogues unblocked)
            for c in range(NCH):
                nc.vector.tensor_reduce(
                    out=xsum_sb[:, c, :],
                    in_=xt_sb[:, c, :].rearrange("p (b t) -> p b t", b=4),
                    axis=AX.X, op=ALU.add,
                )
            push_vproj(0)
            flush()
            push_psb(0)
            push_kproj(1)
            push_psb(1)

            # ---------------- main attention loop ----------------
            for pr in range(PAIRS):
                gp, lpi = pr // 4, pr % 4
                KT_sb, V_sb = kt_of[pr], v_of[gp]
                for J in range(2):
                    jlo, jhi = 2 * J, 2 * J + 1
                    po = [None, None]
                    for hl in range(2):
                        po[hl] = psum.tile([65, 512], F32, tag=f"po{hl}", bufs=1, name=f"po{hl}")
                    for kb in range(jhi + 1):
                        last = kb == jhi
                        dlo = kb == jlo
                        N = 256 if last else 512
                        qoff = 512 * J + (256 if last else 0)
                        for s2 in range(4):
                            pss = psum.tile([128, 2, 512], F32, tag="ss", bufs=2, name="ss")
                            k0 = 512 * kb + 128 * s2
                            for hl in range(2):
                                hsl = slice(64 * hl, 64 * (hl + 1))
                                nc.tensor.matmul(
                                    out=pss[:, hl, 0:N],
                                    lhsT=KT_sb[hsl, k0:k0 + 128],
                                    rhs=QT_all[hsl, pr, qoff:qoff + N],
                                    start=True, stop=True,
                                )
                            e_sb = e_pool.tile([128, 2, 512], BF16, tag="e", name="e_sb")
                            nc.scalar.activation(out=e_sb[:, :, 0:N], in_=pss[:, :, 0:N], func=AF.Exp)
                            if last or dlo:
                                nc.vector.copy_predicated(
                                    out=e_sb[:, :, 0:256],
                                    mask=mdup_sb[:, s2, :, :],
                                    data=wtile[:].rearrange("p (h b) -> p h b", h=2),
                                )
                            for hl in range(2):
                                nc.tensor.matmul(
                                    out=po[hl][:, qoff - 512 * J:qoff - 512 * J + N],
                                    lhsT=V_sb[:, 4 * kb + s2, 2 * lpi + hl, :],
                                    rhs=e_sb[:, hl, 0:N],
                                    start=(kb == 0 and s2 == 0),
                                    stop=(kb == jhi and s2 == 3),
                                    skip_group_check=True,
                                )
                            drain(1)
                    # epilogue: Z, broadcast, numerator correction, divide
                    suf_sb = suf_of[pr]
                    for hl in range(2):
                        hsl = slice(64 * hl, 64 * (hl + 1))
                        zfs = misc_pool.tile([1, 512], F32R, tag="zfs")
                        nc.vector.tensor_add(
                            out=zfs, in0=po[hl][64:65, 0:512],
                            in1=nskrow[:, 2 * J:2 * J + 2, :].rearrange("o a b -> o (a b)"),
                        )
                        zbc = psum.tile([64, 512], F32, tag="pps", bufs=2, name="zbc")
                        nc.tensor.matmul(out=zbc, lhsT=ones_r, rhs=zfs,
                                         start=True, stop=True)
                        rzb = misc_pool.tile([64, 512], F32, tag="rzb")
                        nc.vector.reciprocal_approx_fast(out=rzb, in_=zbc)
                        nm = misc_pool.tile([64, 512], F32, tag="nm")
                        for half, jj in ((0, jlo), (1, jhi)):
                            nc.vector.tensor_scalar_add(
                                out=nm[:, 256 * half:256 * (half + 1)],
                                in0=po[hl][0:64, 256 * half:256 * (half + 1)],
                                scalar1=suf_sb[hsl, jj:jj + 1],
                            )
                        oeng = nc.vector if hl == 0 else nc.gpsimd
                        oeng.tensor_mul(
                            out=O_sb[hsl, pr, 512 * J:512 * (J + 1)],
                            in0=nm, in1=rzb,
                        )
                        drain(4)
                flush()
                if pr + 2 < PAIRS:
                    push_kproj(pr + 2)
                    push_psb(pr + 2)
                if pr == 1:
                    push_vproj(1)

            # ---------------- output projection ----------------
            for dt_ in range(8):
                for J in range(2):
                    ps = psum.tile([128, 512], F32, tag="pps", bufs=2, name="fps")
                    for c in range(NCH):
                        nc.tensor.matmul(
                            out=ps, lhsT=wo_sb[:, c, 128 * dt_:128 * (dt_ + 1)],
                            rhs=O_sb[:, c, 512 * J:512 * (J + 1)],
                            start=(c == 0), stop=(c == NCH - 1),
                        )
                    fo = misc_pool.tile([128, 512], F32, tag="nm", name="fo")
                    nc.vector.tensor_scalar_add(out=fo, in0=ps, scalar1=bocol_sb[:, dt_:dt_ + 1])
                    nc.sync.dma_start(
                        out=outT[128 * dt_:128 * (dt_ + 1), 512 * J:512 * (J + 1)],
                        in_=fo,
                    )
    nc.compile()
    return nc


def qrows_for(p):
    return np.concatenate(
        [np.arange(QT * (2 * j + p), QT * (2 * j + p) + QT) for j in range(NJ)]
    )


def _bf16(a):
    import ml_dtypes
    return np.ascontiguousarray(a.astype(ml_dtypes.bfloat16))


def _chunked(mat2d, inner):
    return np.ascontiguousarray(mat2d.reshape(NCH, 128, inner).transpose(1, 0, 2))


def host_in_maps(x, Wqkv, bqkv, Wo, bo):
    x = np.asarray(x, np.float32)
    Wqkv = np.asarray(Wqkv, np.float32)
    bqkv = np.asarray(bqkv, np.float32)
    Wo = np.asarray(Wo, np.float32)
    bo = np.asarray(bo, np.float32)

    wq = _bf16(_chunked(Wqkv[:, 0:D] * 0.125, D))
    wk = _bf16(_chunked(Wqkv[:, D:2 * D], D))
    wv = _bf16(_chunked(Wqkv[:, 2 * D:3 * D], D))
    wo = _bf16(_chunked(Wo, D))

    b2h = np.empty((128, 16), np.float32)
    b2h[:, 0:8] = bqkv[0:D].reshape(8, 128).T / 8.0
    b2h[:, 8:16] = bqkv[D:2 * D].reshape(8, 128).T
    b2h = np.ascontiguousarray(b2h)
    brow = _bf16(bqkv[2 * D:].reshape(1, D))
    bv512 = np.ascontiguousarray((W_MASK * 512.0 * bqkv[2 * D:].reshape(8, 128)).T)
    bocol = np.ascontiguousarray(bo.reshape(8, 128).T)

    kap = np.arange(128)[:, None]
    r = np.arange(QT)[None, :]
    masks = {}
    for p in range(2):
        mm = np.zeros((128, 4, 1, QT), np.uint8)
        for s in range(4):
            mm[:, s, 0, :] = (128 * s + kap > QT * p + r)   # 1 = masked
        md = np.repeat(mm, 2, axis=2)                        # dup per head
        masks[p] = np.ascontiguousarray(md.reshape(128, 8 * QT))

    in_maps = []
    for core in range(8):
        b, p = core // 2, core % 2
        in_maps.append({
            "xq": _bf16(x[b][qrows_for(p)].T.reshape(NCH, 128, QL).transpose(1, 0, 2)),
            "xt": _bf16(x[b].T.reshape(NCH, 128, S).transpose(1, 0, 2)),
            "wq": wq, "wk": wk, "wv": wv, "wo": wo,
            "b2h": b2h, "brow": brow, "bv512": bv512, "bocol": bocol,
            "mdup": masks[p],
            "onesd": np.ones((1, 64), np.float32),
        })
    return in_maps


_CACHED = {}


def get_program():
    if "nc" not in _CACHED:
        _CACHED["nc"] = build_program()
    return _CACHED["nc"]


def kernel(x, Wqkv, bqkv, Wo, bo):
    from concourse.bass_utils import run_bass_kernel_spmd

    nc = get_program()
    in_maps = host_in_maps(x, Wqkv, bqkv, Wo, bo)
    res = run_bass_kernel_spmd(nc, in_maps, core_ids=list(range(8)))
    out = np.zeros((B, S, D), np.float32)
    for core in range(8):
        b, p = core // 2, core % 2
        out[b, qrows_for(p), :] = res.results[core]["outT"].T
    return out
